# revision 1
# baseline (speedup 1.0000x reference)
"""Trainium2 Bass kernel for nn_AutoEncoder_BNN (8-core data-parallel).

Math (see reference):
    h1 = relu(x @ W_h.T + b_h)
    h2 = relu(h1 @ W_h2.T + b_h2)
    uw = h2 @ W_y.T + b_y;  u, w = uw[:, :N], uw[:, N:]
    Gu = u G^T, Gw = w G^T, Bu = u B^T, Bw = w B^T
    p  = u*Gu + w*Gw + w*Bu - u*Bw + bias_p
    q  = w*Gu - u*Gw - u*Bu - w*Bw + bias_q
    p_out = p @ W_p.T + b_p ; q_out = q @ W_q.T + b_q
    return (uw, p_out, q_out)

Device strategy:
  - Pure data parallel over the batch (1024 samples per core), weights
    replicated.  No collectives.
  - All activations are kept feature-major on chip: actT[feature, batch].
    Every layer is then  outT = W @ inT , which maps to TensorE matmul
    with lhsT = W.T tiles (pre-blocked on host) and rhs = inT tiles.
  - Weights are pre-cast to bf16 and pre-blocked on host into
    [j, p, k, m] with  H[j,p,k,m] = W[j*128+m, k*128+p]  so each
    [128,128] lhsT tile (and each j-strip) is contiguous in DRAM.
  - Karatsuba for the G/B decoder: with s = u+w,
        t1 = G u, t2 = B w, t3 = (G+B) s
        RE = t1 - t2 (= Gu - Bw),  IM = t3 - t1 - t2 (= Gw + Bu)
        p = u*RE + w*IM,  q = w*RE - u*IM
    3 big matmuls instead of 4.
  - bias_p / bias_q are folded into the final biases on host:
        bp_eff = W_p @ bias_p[0] + b_p   (exact, linearity)
  - Two batch passes of 512 columns (one PSUM bank per matmul).
"""

import sys

if "/opt/trn_rl_repo" not in sys.path:
    sys.path.insert(0, "/opt/trn_rl_repo")

import numpy as np
import ml_dtypes

P = 128
BATCH = 8192
N_X = 4096
N_RBF = 1024
N_BUS = 2048
NCORES = 8
NLOC = BATCH // NCORES  # 1024 samples per core
NB = 512                # batch columns per pass (= one PSUM bank of f32)
NPASS = NLOC // NB      # 2

KX = N_X // P    # 32
KR = N_RBF // P  # 8
JB = N_BUS // P  # 16
JUW = 2 * JB     # 32

BF16 = ml_dtypes.bfloat16

_CACHE = {}


def _block_w(W):
    """W [O, I] f32 -> bf16 [J, 128, K, 128] with H[j,p,k,m] = W[j*128+m, k*128+p]."""
    O, I = W.shape
    J, K = O // P, I // P
    return np.ascontiguousarray(
        W.reshape(J, P, K, P).transpose(0, 3, 2, 1).astype(BF16)
    )


def _col_bias(b):
    """b [F] f32 -> [128, F//128] f32 with out[p, j] = b[j*128+p]."""
    return np.ascontiguousarray(b.reshape(-1, P).T.astype(np.float32))


def _build():
    import concourse.tile as tile
    from concourse import bacc, mybir

    f32 = mybir.dt.float32
    bf16 = mybir.dt.bfloat16
    AF = mybir.ActivationFunctionType
    Alu = mybir.AluOpType

    nc = bacc.Bacc("TRN2", target_bir_lowering=False, debug=False)

    xT_d = nc.dram_tensor("xT", [P, KX, NLOC], bf16, kind="ExternalInput")
    Wh_d = nc.dram_tensor("Wh", [KR, P, KX, P], bf16, kind="ExternalInput")
    bh_d = nc.dram_tensor("bh", [P, KR], f32, kind="ExternalInput")
    Wh2_d = nc.dram_tensor("Wh2", [KR, P, KR, P], bf16, kind="ExternalInput")
    bh2_d = nc.dram_tensor("bh2", [P, KR], f32, kind="ExternalInput")
    Wy_d = nc.dram_tensor("Wy", [JUW, P, KR, P], bf16, kind="ExternalInput")
    by_d = nc.dram_tensor("by", [P, JUW], f32, kind="ExternalInput")
    Gt_d = nc.dram_tensor("Gt", [JB, P, JB, P], bf16, kind="ExternalInput")
    Bt_d = nc.dram_tensor("Bt", [JB, P, JB, P], bf16, kind="ExternalInput")
    GBt_d = nc.dram_tensor("GBt", [JB, P, JB, P], bf16, kind="ExternalInput")
    Wp_d = nc.dram_tensor("Wp", [JB, P, JB, P], bf16, kind="ExternalInput")
    bp_d = nc.dram_tensor("bp", [P, JB], f32, kind="ExternalInput")
    Wq_d = nc.dram_tensor("Wq", [JB, P, JB, P], bf16, kind="ExternalInput")
    bq_d = nc.dram_tensor("bq", [P, JB], f32, kind="ExternalInput")

    uw_o = nc.dram_tensor("uw_o", [JUW, P, NLOC], bf16, kind="ExternalOutput")
    p_o = nc.dram_tensor("p_o", [JB, P, NLOC], bf16, kind="ExternalOutput")
    q_o = nc.dram_tensor("q_o", [JB, P, NLOC], bf16, kind="ExternalOutput")

    with tile.TileContext(nc) as tc:
        with tc.tile_pool(name="bias", bufs=1) as biasp:
            bh_sb = biasp.tile([P, KR], f32, tag="bh")
            bh2_sb = biasp.tile([P, KR], f32, tag="bh2")
            by_sb = biasp.tile([P, JUW], f32, tag="by")
            bp_sb = biasp.tile([P, JB], f32, tag="bp")
            bq_sb = biasp.tile([P, JB], f32, tag="bq")
            nc.sync.dma_start(bh_sb[:], bh_d[:])
            nc.sync.dma_start(bh2_sb[:], bh2_d[:])
            nc.sync.dma_start(by_sb[:], by_d[:])
            nc.sync.dma_start(bp_sb[:], bp_d[:])
            nc.sync.dma_start(bq_sb[:], bq_d[:])

            for ip in range(NPASS):
                ns = slice(ip * NB, (ip + 1) * NB)
                with tc.tile_pool(name=f"acts{ip}", bufs=1) as acts:
                    # ---- phase A: h1 = relu(Wh @ xT + bh) ----
                    x_sb = acts.tile([P, KX, NB], bf16, tag="x")
                    nc.sync.dma_start(x_sb[:], xT_d[:, :, ns])
                    h1_sb = acts.tile([P, KR, NB], bf16, tag="h1")
                    with (
                        tc.tile_pool(name=f"wA{ip}", bufs=2) as wA,
                        tc.tile_pool(name=f"psA{ip}", bufs=4, space="PSUM") as psA,
                    ):
                        for j in range(KR):
                            strip = wA.tile([P, KX, P], bf16, tag="w")
                            nc.sync.dma_start(strip[:], Wh_d[j])
                            ps = psA.tile([P, NB], f32, tag="ps")
                            for k in range(KX):
                                nc.tensor.matmul(
                                    ps[:], strip[:, k, :], x_sb[:, k, :],
                                    start=(k == 0), stop=(k == KX - 1),
                                )
                            nc.scalar.activation(
                                h1_sb[:, j, :], ps[:], AF.Relu,
                                bias=bh_sb[:, j : j + 1],
                            )

                    # ---- phase B: h2 = relu(Wh2 @ h1 + bh2) ----
                    h2_sb = acts.tile([P, KR, NB], bf16, tag="h2")
                    with (
                        tc.tile_pool(name=f"wB{ip}", bufs=2) as wB,
                        tc.tile_pool(name=f"psB{ip}", bufs=4, space="PSUM") as psB,
                    ):
                        for j in range(KR):
                            strip = wB.tile([P, KR, P], bf16, tag="w")
                            nc.sync.dma_start(strip[:], Wh2_d[j])
                            ps = psB.tile([P, NB], f32, tag="ps")
                            for k in range(KR):
                                nc.tensor.matmul(
                                    ps[:], strip[:, k, :], h1_sb[:, k, :],
                                    start=(k == 0), stop=(k == KR - 1),
                                )
                            nc.scalar.activation(
                                h2_sb[:, j, :], ps[:], AF.Relu,
                                bias=bh2_sb[:, j : j + 1],
                            )

                    # ---- phase C: uw = Wy @ h2 + by  (also DMA out) ----
                    uw_sb = acts.tile([P, JUW, NB], bf16, tag="uw")
                    with (
                        tc.tile_pool(name=f"wC{ip}", bufs=3) as wC,
                        tc.tile_pool(name=f"psC{ip}", bufs=4, space="PSUM") as psC,
                    ):
                        for j in range(JUW):
                            strip = wC.tile([P, KR, P], bf16, tag="w")
                            nc.sync.dma_start(strip[:], Wy_d[j])
                            ps = psC.tile([P, NB], f32, tag="ps")
                            for k in range(KR):
                                nc.tensor.matmul(
                                    ps[:], strip[:, k, :], h2_sb[:, k, :],
                                    start=(k == 0), stop=(k == KR - 1),
                                )
                            nc.scalar.activation(
                                uw_sb[:, j, :], ps[:], AF.Identity,
                                bias=by_sb[:, j : j + 1],
                            )
                            nc.sync.dma_start(uw_o[j][:, ns], uw_sb[:, j, :])

                    # ---- s = u + w ----
                    s_sb = acts.tile([P, JB, NB], bf16, tag="s")
                    for k in range(JB):
                        nc.vector.tensor_tensor(
                            s_sb[:, k, :], uw_sb[:, k, :], uw_sb[:, JB + k, :],
                            Alu.add,
                        )

                    # ---- phase D: t1=G u, t2=B w, t3=(G+B) s; p,q elementwise ----
                    p_sb = acts.tile([P, JB, NB], bf16, tag="p")
                    q_sb = acts.tile([P, JB, NB], bf16, tag="q")
                    with (
                        tc.tile_pool(name=f"wD{ip}", bufs=2) as wD,
                        tc.tile_pool(name=f"psD{ip}", bufs=2, space="PSUM") as psD,
                        tc.tile_pool(name=f"tmpD{ip}", bufs=3) as tmpD,
                    ):
                        for j in range(JB):
                            gs = wD.tile([P, JB, P], bf16, tag="g")
                            nc.sync.dma_start(gs[:], Gt_d[j])
                            bs = wD.tile([P, JB, P], bf16, tag="b")
                            nc.sync.dma_start(bs[:], Bt_d[j])
                            gbs = wD.tile([P, JB, P], bf16, tag="gb")
                            nc.sync.dma_start(gbs[:], GBt_d[j])

                            ps1 = psD.tile([P, NB], f32, tag="ps1")
                            for k in range(JB):
                                nc.tensor.matmul(
                                    ps1[:], gs[:, k, :], uw_sb[:, k, :],
                                    start=(k == 0), stop=(k == JB - 1),
                                )
                            ps2 = psD.tile([P, NB], f32, tag="ps2")
                            for k in range(JB):
                                nc.tensor.matmul(
                                    ps2[:], bs[:, k, :], uw_sb[:, JB + k, :],
                                    start=(k == 0), stop=(k == JB - 1),
                                )
                            ps3 = psD.tile([P, NB], f32, tag="ps3")
                            for k in range(JB):
                                nc.tensor.matmul(
                                    ps3[:], gbs[:, k, :], s_sb[:, k, :],
                                    start=(k == 0), stop=(k == JB - 1),
                                )

                            t1 = tmpD.tile([P, NB], bf16, tag="t1")
                            nc.scalar.copy(t1[:], ps1[:])
                            t2 = tmpD.tile([P, NB], bf16, tag="t2")
                            nc.scalar.copy(t2[:], ps2[:])
                            t3 = tmpD.tile([P, NB], bf16, tag="t3")
                            nc.scalar.copy(t3[:], ps3[:])

                            re = tmpD.tile([P, NB], bf16, tag="re")
                            nc.vector.tensor_tensor(re[:], t1[:], t2[:], Alu.subtract)
                            im = tmpD.tile([P, NB], bf16, tag="im")
                            nc.vector.tensor_tensor(im[:], t3[:], t1[:], Alu.subtract)
                            nc.vector.tensor_tensor(im[:], im[:], t2[:], Alu.subtract)

                            m1 = tmpD.tile([P, NB], bf16, tag="m1")
                            nc.vector.tensor_tensor(m1[:], uw_sb[:, j, :], re[:], Alu.mult)
                            m2 = tmpD.tile([P, NB], bf16, tag="m2")
                            nc.vector.tensor_tensor(m2[:], uw_sb[:, JB + j, :], im[:], Alu.mult)
                            nc.vector.tensor_tensor(p_sb[:, j, :], m1[:], m2[:], Alu.add)

                            m3 = tmpD.tile([P, NB], bf16, tag="m3")
                            nc.vector.tensor_tensor(m3[:], uw_sb[:, JB + j, :], re[:], Alu.mult)
                            m4 = tmpD.tile([P, NB], bf16, tag="m4")
                            nc.vector.tensor_tensor(m4[:], uw_sb[:, j, :], im[:], Alu.mult)
                            nc.vector.tensor_tensor(q_sb[:, j, :], m3[:], m4[:], Alu.subtract)

                    # ---- phase E: p_out = Wp @ p + bp ; q_out = Wq @ q + bq ----
                    with (
                        tc.tile_pool(name=f"wE{ip}", bufs=2) as wE,
                        tc.tile_pool(name=f"psE{ip}", bufs=2, space="PSUM") as psE,
                        tc.tile_pool(name=f"outE{ip}", bufs=3) as outE,
                    ):
                        for j in range(JB):
                            wps = wE.tile([P, JB, P], bf16, tag="wp")
                            nc.sync.dma_start(wps[:], Wp_d[j])
                            ps = psE.tile([P, NB], f32, tag="pp")
                            for k in range(JB):
                                nc.tensor.matmul(
                                    ps[:], wps[:, k, :], p_sb[:, k, :],
                                    start=(k == 0), stop=(k == JB - 1),
                                )
                            po = outE.tile([P, NB], bf16, tag="po")
                            nc.scalar.activation(
                                po[:], ps[:], AF.Identity, bias=bp_sb[:, j : j + 1]
                            )
                            nc.sync.dma_start(p_o[j][:, ns], po[:])

                            wqs = wE.tile([P, JB, P], bf16, tag="wq")
                            nc.sync.dma_start(wqs[:], Wq_d[j])
                            ps2 = psE.tile([P, NB], f32, tag="pq")
                            for k in range(JB):
                                nc.tensor.matmul(
                                    ps2[:], wqs[:, k, :], q_sb[:, k, :],
                                    start=(k == 0), stop=(k == JB - 1),
                                )
                            qo = outE.tile([P, NB], bf16, tag="qo")
                            nc.scalar.activation(
                                qo[:], ps2[:], AF.Identity, bias=bq_sb[:, j : j + 1]
                            )
                            nc.sync.dma_start(q_o[j][:, ns], qo[:])

    nc.compile()
    return nc


def _get_nc():
    if "nc" not in _CACHE:
        _CACHE["nc"] = _build()
    return _CACHE["nc"]


def _prep_shared(W_h, b_h, W_h2, b_h2, W_y, b_y, G, Bm, bias_p, bias_q,
                 W_p, b_p, W_q, b_q):
    f = np.float32
    W_h = np.asarray(W_h, f)
    W_h2 = np.asarray(W_h2, f)
    W_y = np.asarray(W_y, f)
    G = np.asarray(G, f)
    Bm = np.asarray(Bm, f)
    W_p = np.asarray(W_p, f)
    W_q = np.asarray(W_q, f)
    bias_p = np.asarray(bias_p, f).reshape(-1)
    bias_q = np.asarray(bias_q, f).reshape(-1)
    bp_eff = W_p @ bias_p + np.asarray(b_p, f)
    bq_eff = W_q @ bias_q + np.asarray(b_q, f)
    return {
        "Wh": _block_w(W_h),
        "bh": _col_bias(np.asarray(b_h, f)),
        "Wh2": _block_w(W_h2),
        "bh2": _col_bias(np.asarray(b_h2, f)),
        "Wy": _block_w(W_y),
        "by": _col_bias(np.asarray(b_y, f)),
        "Gt": _block_w(G),
        "Bt": _block_w(Bm),
        "GBt": _block_w(G + Bm),
        "Wp": _block_w(W_p),
        "bp": _col_bias(bp_eff),
        "Wq": _block_w(W_q),
        "bq": _col_bias(bq_eff),
    }


def _prep_x_shard(x, c):
    xs = np.asarray(x[c * NLOC : (c + 1) * NLOC], np.float32)  # [NLOC, N_X]
    # want [p, k, n] with val = xs[n, k*128+p]
    return np.ascontiguousarray(
        xs.T.reshape(KX, P, NLOC).transpose(1, 0, 2).astype(BF16)
    )


def run_spmd(inputs, trace=False, **kw):
    """Build in_maps, run on 8 cores, return (results_list, exec_time_ns)."""
    from concourse.bass_utils import run_bass_kernel_spmd

    nc = _get_nc()
    x = inputs["x"]
    shared = _prep_shared(
        inputs["W_h"], inputs["b_h"], inputs["W_h2"], inputs["b_h2"],
        inputs["W_y"], inputs["b_y"], inputs["G"], inputs["Bm"],
        inputs["bias_p"], inputs["bias_q"],
        inputs["W_p"], inputs["b_p"], inputs["W_q"], inputs["b_q"],
    )
    in_maps = [dict(shared, xT=_prep_x_shard(x, c)) for c in range(NCORES)]
    res = run_bass_kernel_spmd(nc, in_maps, core_ids=list(range(NCORES)),
                               trace=trace, **kw)
    return res


def kernel(**inputs):
    res = run_spmd(inputs)
    uw_parts, p_parts, q_parts = [], [], []
    for c in range(NCORES):
        r = res.results[c]
        uw_parts.append(
            np.asarray(r["uw_o"]).astype(np.float32).reshape(2 * N_BUS, NLOC).T
        )
        p_parts.append(
            np.asarray(r["p_o"]).astype(np.float32).reshape(N_BUS, NLOC).T
        )
        q_parts.append(
            np.asarray(r["q_o"]).astype(np.float32).reshape(N_BUS, NLOC).T
        )
    uw = np.concatenate(uw_parts, axis=0)
    p = np.concatenate(p_parts, axis=0)
    q = np.concatenate(q_parts, axis=0)
    return (uw, p, q)


# revision 3
# speedup vs baseline: 1.1256x; 1.1256x over previous
"""Trainium2 Bass kernel for nn_AutoEncoder_BNN (8-core data-parallel).

Math (see reference):
    h1 = relu(x @ W_h.T + b_h)
    h2 = relu(h1 @ W_h2.T + b_h2)
    uw = h2 @ W_y.T + b_y;  u, w = uw[:, :N], uw[:, N:]
    Gu = u G^T, Gw = w G^T, Bu = u B^T, Bw = w B^T
    p  = u*Gu + w*Gw + w*Bu - u*Bw + bias_p
    q  = w*Gu - u*Gw - u*Bu - w*Bw + bias_q
    p_out = p @ W_p.T + b_p ; q_out = q @ W_q.T + b_q
    return (uw, p_out, q_out)

Device strategy:
  - Pure data parallel over the batch (1024 samples per core), weights
    replicated.  No collectives.
  - All activations are kept feature-major on chip: actT[feature, batch].
    Every layer is then  outT = W @ inT , which maps to TensorE matmul
    with lhsT = W.T tiles (pre-blocked on host) and rhs = inT tiles.
  - Weights are pre-cast to bf16 and pre-blocked on host so each lhsT
    strip is contiguous in DRAM.
  - Karatsuba for the G/B decoder: with s = u+w,
        t1 = G u, t2 = B w, t3 = (G+B) s
        RE = t1 - t2 (= Gu - Bw),  IM = t3 - t1 - t2 (= Gw + Bu)
        p = u*RE + w*IM,  q = w*RE - u*IM
    3 big matmuls instead of 4.
  - bias_p / bias_q are folded into the final biases on host:
        bp_eff = W_p @ bias_p[0] + b_p   (exact, linearity)
  - Two batch passes of 512 columns (one PSUM bank per matmul), with
    persistent tile pools so DMA prefetch flows across phase/pass
    boundaries and the PE never starves.
"""

import sys

if "/opt/trn_rl_repo" not in sys.path:
    sys.path.insert(0, "/opt/trn_rl_repo")

import numpy as np
import ml_dtypes

P = 128
BATCH = 8192
N_X = 4096
N_RBF = 1024
N_BUS = 2048
NCORES = 8
NLOC = BATCH // NCORES  # 1024 samples per core
NB = 512                # batch columns per pass (= one PSUM bank of f32)
NPASS = NLOC // NB      # 2

KX = N_X // P    # 32
KR = N_RBF // P  # 8
JB = N_BUS // P  # 16
JUW = 2 * JB     # 32

BF16 = ml_dtypes.bfloat16

_CACHE = {}


def _block_w(W):
    """W [O, I] f32 -> bf16 [J, 128, K, 128] with H[j,p,k,m] = W[j*128+m, k*128+p].

    Strip H[j] is the lhsT for output tile j: contiguous per partition."""
    O, I = W.shape
    J, K = O // P, I // P
    return np.ascontiguousarray(
        W.reshape(J, P, K, P).transpose(0, 3, 2, 1).astype(BF16)
    )


def _block_w_k(W):
    """W [O, I] f32 -> bf16 [K, 128, J, 128] with H[k,p,j,m] = W[j*128+m, k*128+p].

    Strip H[k] holds the lhsT tiles of every output tile j for one k."""
    O, I = W.shape
    J, K = O // P, I // P
    return np.ascontiguousarray(
        W.reshape(J, P, K, P).transpose(2, 3, 0, 1).astype(BF16)
    )


def _col_bias(b):
    """b [F] f32 -> [128, F//128] f32 with out[p, j] = b[j*128+p]."""
    return np.ascontiguousarray(b.reshape(-1, P).T.astype(np.float32))


def _build():
    import concourse.tile as tile
    from concourse import bacc, mybir

    f32 = mybir.dt.float32
    bf16 = mybir.dt.bfloat16
    AF = mybir.ActivationFunctionType
    Alu = mybir.AluOpType

    nc = bacc.Bacc("TRN2", target_bir_lowering=False, debug=False)

    xT_d = nc.dram_tensor("xT", [NPASS, KX, P, NB], bf16, kind="ExternalInput")
    Wh_d = nc.dram_tensor("Wh", [KX, P, KR, P], bf16, kind="ExternalInput")
    bh_d = nc.dram_tensor("bh", [P, KR], f32, kind="ExternalInput")
    Wh2_d = nc.dram_tensor("Wh2", [KR, P, KR, P], bf16, kind="ExternalInput")
    bh2_d = nc.dram_tensor("bh2", [P, KR], f32, kind="ExternalInput")
    Wy_d = nc.dram_tensor("Wy", [JUW, P, KR, P], bf16, kind="ExternalInput")
    by_d = nc.dram_tensor("by", [P, JUW], f32, kind="ExternalInput")
    Gt_d = nc.dram_tensor("Gt", [JB, P, JB, P], bf16, kind="ExternalInput")
    Bt_d = nc.dram_tensor("Bt", [JB, P, JB, P], bf16, kind="ExternalInput")
    GBt_d = nc.dram_tensor("GBt", [JB, P, JB, P], bf16, kind="ExternalInput")
    Wp_d = nc.dram_tensor("Wp", [JB, P, JB, P], bf16, kind="ExternalInput")
    bp_d = nc.dram_tensor("bp", [P, JB], f32, kind="ExternalInput")
    Wq_d = nc.dram_tensor("Wq", [JB, P, JB, P], bf16, kind="ExternalInput")
    bq_d = nc.dram_tensor("bq", [P, JB], f32, kind="ExternalInput")

    uw_o = nc.dram_tensor("uw_o", [JUW, P, NLOC], bf16, kind="ExternalOutput")
    p_o = nc.dram_tensor("p_o", [JB, P, NLOC], bf16, kind="ExternalOutput")
    q_o = nc.dram_tensor("q_o", [JB, P, NLOC], bf16, kind="ExternalOutput")

    with tile.TileContext(nc) as tc:
        with (
            tc.tile_pool(name="bias", bufs=1) as biasp,
            tc.tile_pool(name="xk", bufs=6) as xkp,
            tc.tile_pool(name="h1", bufs=1) as h1p,
            tc.tile_pool(name="h2", bufs=1) as h2p,
            tc.tile_pool(name="uw", bufs=1) as uwp,
            tc.tile_pool(name="spq", bufs=1) as spqp,
            tc.tile_pool(name="w", bufs=8) as wpool,
            tc.tile_pool(name="ps", bufs=8, space="PSUM") as psp,
            tc.tile_pool(name="tmp", bufs=2) as tmpp,
            tc.tile_pool(name="oute", bufs=4) as outp,
        ):
            bh_sb = biasp.tile([P, KR], f32, tag="bh")
            bh2_sb = biasp.tile([P, KR], f32, tag="bh2")
            by_sb = biasp.tile([P, JUW], f32, tag="by")
            bp_sb = biasp.tile([P, JB], f32, tag="bp")
            bq_sb = biasp.tile([P, JB], f32, tag="bq")
            nc.sync.dma_start(bh_sb[:], bh_d[:])
            nc.sync.dma_start(bh2_sb[:], bh2_d[:])
            nc.sync.dma_start(by_sb[:], by_d[:])
            nc.sync.dma_start(bp_sb[:], bp_d[:])
            nc.sync.dma_start(bq_sb[:], bq_d[:])

            for ip in range(NPASS):
                ns = slice(ip * NB, (ip + 1) * NB)

                # ---- phase A: h1 = relu(Wh @ xT + bh), k-major over 8 banks --
                h1_sb = h1p.tile([P, KR, NB], bf16, tag="h1")
                ps_a = [
                    psp.tile([P, NB], f32, tag="ps", name=f"ps_a{ip}_{j}")
                    for j in range(KR)
                ]
                for k in range(KX):
                    xk = xkp.tile([P, NB], bf16, tag="xk")
                    nc.sync.dma_start(xk[:], xT_d[ip, k])
                    strip = wpool.tile([P, KR, P], bf16, tag="w")
                    nc.sync.dma_start(strip[:], Wh_d[k])
                    for j in range(KR):
                        nc.tensor.matmul(
                            ps_a[j][:], strip[:, j, :], xk[:],
                            start=(k == 0), stop=(k == KX - 1),
                        )
                for j in range(KR):
                    nc.scalar.activation(
                        h1_sb[:, j, :], ps_a[j][:], AF.Relu,
                        bias=bh_sb[:, j : j + 1],
                    )

                # ---- phase B: h2 = relu(Wh2 @ h1 + bh2) ----
                h2_sb = h2p.tile([P, KR, NB], bf16, tag="h2")
                for j in range(KR):
                    strip = wpool.tile([P, KR, P], bf16, tag="w")
                    nc.sync.dma_start(strip[:], Wh2_d[j])
                    ps = psp.tile([P, NB], f32, tag="ps")
                    for k in range(KR):
                        nc.tensor.matmul(
                            ps[:], strip[:, k, :], h1_sb[:, k, :],
                            start=(k == 0), stop=(k == KR - 1),
                        )
                    nc.scalar.activation(
                        h2_sb[:, j, :], ps[:], AF.Relu,
                        bias=bh2_sb[:, j : j + 1],
                    )

                # ---- phase C: uw = Wy @ h2 + by  (also DMA out) ----
                uw_sb = uwp.tile([P, JUW, NB], bf16, tag="uw")
                for j in range(JUW):
                    strip = wpool.tile([P, KR, P], bf16, tag="w")
                    nc.sync.dma_start(strip[:], Wy_d[j])
                    ps = psp.tile([P, NB], f32, tag="ps")
                    for k in range(KR):
                        nc.tensor.matmul(
                            ps[:], strip[:, k, :], h2_sb[:, k, :],
                            start=(k == 0), stop=(k == KR - 1),
                        )
                    nc.scalar.activation(
                        uw_sb[:, j, :], ps[:], AF.Identity,
                        bias=by_sb[:, j : j + 1],
                    )
                    nc.sync.dma_start(uw_o[j][:, ns], uw_sb[:, j, :])

                # ---- s = u + w ----
                s_sb = spqp.tile([P, JB, NB], bf16, tag="s")
                for k in range(JB):
                    nc.vector.tensor_tensor(
                        s_sb[:, k, :], uw_sb[:, k, :], uw_sb[:, JB + k, :],
                        Alu.add,
                    )

                # ---- phase D: t1=G u, t2=B w, t3=(G+B) s; p,q elementwise ----
                p_sb = spqp.tile([P, JB, NB], bf16, tag="p")
                q_sb = spqp.tile([P, JB, NB], bf16, tag="q")
                for j in range(JB):
                    gs = wpool.tile([P, JB, P], bf16, tag="w")
                    nc.sync.dma_start(gs[:], Gt_d[j])
                    bs = wpool.tile([P, JB, P], bf16, tag="w")
                    nc.sync.dma_start(bs[:], Bt_d[j])
                    gbs = wpool.tile([P, JB, P], bf16, tag="w")
                    nc.sync.dma_start(gbs[:], GBt_d[j])

                    ps1 = psp.tile([P, NB], f32, tag="ps")
                    for k in range(JB):
                        nc.tensor.matmul(
                            ps1[:], gs[:, k, :], uw_sb[:, k, :],
                            start=(k == 0), stop=(k == JB - 1),
                        )
                    ps2 = psp.tile([P, NB], f32, tag="ps")
                    for k in range(JB):
                        nc.tensor.matmul(
                            ps2[:], bs[:, k, :], uw_sb[:, JB + k, :],
                            start=(k == 0), stop=(k == JB - 1),
                        )
                    ps3 = psp.tile([P, NB], f32, tag="ps")
                    for k in range(JB):
                        nc.tensor.matmul(
                            ps3[:], gbs[:, k, :], s_sb[:, k, :],
                            start=(k == 0), stop=(k == JB - 1),
                        )

                    t1 = tmpp.tile([P, NB], bf16, tag="t1")
                    nc.scalar.copy(t1[:], ps1[:])
                    t2 = tmpp.tile([P, NB], bf16, tag="t2")
                    nc.scalar.copy(t2[:], ps2[:])
                    t3 = tmpp.tile([P, NB], bf16, tag="t3")
                    nc.scalar.copy(t3[:], ps3[:])

                    re = tmpp.tile([P, NB], bf16, tag="re")
                    nc.vector.tensor_tensor(re[:], t1[:], t2[:], Alu.subtract)
                    im = tmpp.tile([P, NB], bf16, tag="im")
                    nc.vector.tensor_tensor(im[:], t3[:], t1[:], Alu.subtract)
                    nc.vector.tensor_tensor(im[:], im[:], t2[:], Alu.subtract)

                    m1 = tmpp.tile([P, NB], bf16, tag="m1")
                    nc.vector.tensor_tensor(m1[:], uw_sb[:, j, :], re[:], Alu.mult)
                    m2 = tmpp.tile([P, NB], bf16, tag="m2")
                    nc.vector.tensor_tensor(m2[:], uw_sb[:, JB + j, :], im[:], Alu.mult)
                    nc.vector.tensor_tensor(p_sb[:, j, :], m1[:], m2[:], Alu.add)

                    m3 = tmpp.tile([P, NB], bf16, tag="m3")
                    nc.vector.tensor_tensor(m3[:], uw_sb[:, JB + j, :], re[:], Alu.mult)
                    m4 = tmpp.tile([P, NB], bf16, tag="m4")
                    nc.vector.tensor_tensor(m4[:], uw_sb[:, j, :], im[:], Alu.mult)
                    nc.vector.tensor_tensor(q_sb[:, j, :], m3[:], m4[:], Alu.subtract)

                # ---- phase E: p_out = Wp @ p + bp ; q_out = Wq @ q + bq ----
                for j in range(JB):
                    wps = wpool.tile([P, JB, P], bf16, tag="w")
                    nc.sync.dma_start(wps[:], Wp_d[j])
                    ps = psp.tile([P, NB], f32, tag="ps")
                    for k in range(JB):
                        nc.tensor.matmul(
                            ps[:], wps[:, k, :], p_sb[:, k, :],
                            start=(k == 0), stop=(k == JB - 1),
                        )
                    po = outp.tile([P, NB], bf16, tag="po")
                    nc.scalar.activation(
                        po[:], ps[:], AF.Identity, bias=bp_sb[:, j : j + 1]
                    )
                    nc.sync.dma_start(p_o[j][:, ns], po[:])

                    wqs = wpool.tile([P, JB, P], bf16, tag="w")
                    nc.sync.dma_start(wqs[:], Wq_d[j])
                    ps2 = psp.tile([P, NB], f32, tag="ps")
                    for k in range(JB):
                        nc.tensor.matmul(
                            ps2[:], wqs[:, k, :], q_sb[:, k, :],
                            start=(k == 0), stop=(k == JB - 1),
                        )
                    qo = outp.tile([P, NB], bf16, tag="qo")
                    nc.scalar.activation(
                        qo[:], ps2[:], AF.Identity, bias=bq_sb[:, j : j + 1]
                    )
                    nc.sync.dma_start(q_o[j][:, ns], qo[:])

    nc.compile()
    return nc


def _get_nc():
    if "nc" not in _CACHE:
        _CACHE["nc"] = _build()
    return _CACHE["nc"]


def _prep_shared(W_h, b_h, W_h2, b_h2, W_y, b_y, G, Bm, bias_p, bias_q,
                 W_p, b_p, W_q, b_q):
    f = np.float32
    W_h = np.asarray(W_h, f)
    W_h2 = np.asarray(W_h2, f)
    W_y = np.asarray(W_y, f)
    G = np.asarray(G, f)
    Bm = np.asarray(Bm, f)
    W_p = np.asarray(W_p, f)
    W_q = np.asarray(W_q, f)
    bias_p = np.asarray(bias_p, f).reshape(-1)
    bias_q = np.asarray(bias_q, f).reshape(-1)
    bp_eff = W_p @ bias_p + np.asarray(b_p, f)
    bq_eff = W_q @ bias_q + np.asarray(b_q, f)
    return {
        "Wh": _block_w_k(W_h),
        "bh": _col_bias(np.asarray(b_h, f)),
        "Wh2": _block_w(W_h2),
        "bh2": _col_bias(np.asarray(b_h2, f)),
        "Wy": _block_w(W_y),
        "by": _col_bias(np.asarray(b_y, f)),
        "Gt": _block_w(G),
        "Bt": _block_w(Bm),
        "GBt": _block_w(G + Bm),
        "Wp": _block_w(W_p),
        "bp": _col_bias(bp_eff),
        "Wq": _block_w(W_q),
        "bq": _col_bias(bq_eff),
    }


def _prep_x_shard(x, c):
    xs = np.asarray(x[c * NLOC : (c + 1) * NLOC], np.float32)  # [NLOC, N_X]
    # want [ip, k, p, n] with val = xs[ip*NB + n, k*128 + p]
    return np.ascontiguousarray(
        xs.T.reshape(KX, P, NPASS, NB).transpose(2, 0, 1, 3).astype(BF16)
    )


def run_spmd(inputs, trace=False, **kw):
    """Build in_maps, run on 8 cores, return BassKernelResults."""
    from concourse.bass_utils import run_bass_kernel_spmd

    nc = _get_nc()
    x = inputs["x"]
    shared = _prep_shared(
        inputs["W_h"], inputs["b_h"], inputs["W_h2"], inputs["b_h2"],
        inputs["W_y"], inputs["b_y"], inputs["G"], inputs["Bm"],
        inputs["bias_p"], inputs["bias_q"],
        inputs["W_p"], inputs["b_p"], inputs["W_q"], inputs["b_q"],
    )
    in_maps = [dict(shared, xT=_prep_x_shard(x, c)) for c in range(NCORES)]
    res = run_bass_kernel_spmd(nc, in_maps, core_ids=list(range(NCORES)),
                               trace=trace, **kw)
    return res


def kernel(**inputs):
    res = run_spmd(inputs)
    uw_parts, p_parts, q_parts = [], [], []
    for c in range(NCORES):
        r = res.results[c]
        uw_parts.append(
            np.asarray(r["uw_o"]).astype(np.float32).reshape(2 * N_BUS, NLOC).T
        )
        p_parts.append(
            np.asarray(r["p_o"]).astype(np.float32).reshape(N_BUS, NLOC).T
        )
        q_parts.append(
            np.asarray(r["q_o"]).astype(np.float32).reshape(N_BUS, NLOC).T
        )
    uw = np.concatenate(uw_parts, axis=0)
    p = np.concatenate(p_parts, axis=0)
    q = np.concatenate(q_parts, axis=0)
    return (uw, p, q)
